# revision 1
# baseline (speedup 1.0000x reference)
"""Multi-head attention (B=512,S=64,D=1024,H=16) on 8 trn2 NeuronCores.

Strategy: pure data-parallel over the batch dim — each core gets 64 batches
(4096 tokens) and runs the full fused MHA layer locally; no collectives.

Per-core dataflow (token chunks of 512 = 8 batches):
  xT [1024, tok] bf16 arrives pre-transposed (feature-major) from the host
  (one strided dma_start per chunk; no on-chip x transposes)
  qT = Wq.T @ xT, kT = Wk.T @ xT     (feature-major)
  v  = x @ Wv                        (token-major, interleaved with ones col)
  scoresT[k,q] = (kT slice).T @ (qT slice)   per (batch,head), quadrant packed
  expS = exp(scoresT/32)             (no max-subtract: logits are ~N(0,0.1))
  ctx[q,:]|sumexp[q] = expS.T @ [v|1]        -> normalize with per-partition
  recip; ctxT via PE transposes batched 4-to-a-PSUM-bank with a single
  strided DVE drain; out = gelu(ctx @ Wo) token-major -> DRAM.

Weights/x are converted to bf16 on the host (halves input DMA) and loaded
with one merged dma_start per matrix (quarters for wq/x(0) so the first
projection matmuls start while the rest is in flight). The PE has a
HAM power budget (~536us of full-speed execution from first sustained
activity, then 4/8 duty-cycling) — every saved PE cycle or mid-stream
stall is worth ~1.75x in wall time once the budget is exhausted.

(A block-diagonal attention variant with half the attention matmuls was
tried and measured SLOWER on HW — finer-grained units pipelined worse —
so the quadrant scheme below stands.)

The emission order software-pipelines chunks: window ch emits chunk ch's
dense QKV projections interleaved with chunk ch-1's attention and chunk
ch-3's output projection (the delay keeps the tail full of dense matmuls).

PSUM packing rule (hardware): two concurrent matmuls may share a PSUM bank
only if they use the same array row-strip (same operand base partition) or
a strict diagonal (row,col) placement; different row-strips draining into
one bank is fatal. All sharing here uses full-128 row strips.
PSUM budget (bank-granular): proj 2 + sc 4 + cx 2 = 8 banks (the deep
score rotation matches the score->exp->ctx lookahead; measured -19us going
from sc 2 to sc 3). ctx transposes run on the xbar DMA engine, not the PE.
"""

import sys

sys.path.insert(0, "/opt/trn_rl_repo")

import numpy as np
import ml_dtypes

import concourse.bass as bass
import concourse.tile as tile
from concourse import mybir
from concourse.bass_utils import run_bass_kernel_spmd
from concourse.masks import make_identity

F32 = mybir.dt.float32
BF = mybir.dt.bfloat16

B, S, D, H = 512, 64, 1024, 16
DH = D // H  # 64
NCORES = 8
BL = B // NCORES  # 64 batches per core
NTOK = BL * S  # 4096 tokens per core
CHUNK = 512  # tokens per pipeline chunk (8 batches)
NCH = NTOK // CHUNK  # 8
TT = CHUNK // 128  # 4 token-tiles per chunk
KT = D // 128  # 8 d-tiles
SCALE = 1.0 / np.sqrt(np.float32(D))  # 1/32


def _split_multiwait(nc, limit=1):
    """walrus can emit at most one sync-wait per instruction; TileContext's
    tail drain carries one wait per touched processor. Hoist extras onto
    chained NOPs."""
    f = nc.m.functions[0]
    for blk in f.blocks:
        new_insts = []
        for inst in blk.instructions:
            si = inst.sync_info
            if si is not None and len(si.on_wait) > limit:
                extra = si.on_wait[:-limit]
                keep = si.on_wait[-limit:]
                for i, w in enumerate(extra):
                    nop = mybir.InstNoOp(
                        name=f"{inst.name}-waitsplit{i}",
                        sync_info=mybir.SyncInfo(on_wait=[w], on_update=[]),
                        bass_nofuse=True,
                        ins=[],
                        outs=[],
                    )
                    nop.engine = inst.engine
                    new_insts.append(nop)
                si.on_wait[:] = keep
            new_insts.append(inst)
        blk.instructions[:] = new_insts


def _interleave(a, b, w=1.0):
    """Merge two unit lists round-robin, proportionally to their lengths.
    w > 1 front-loads list a (a's units are emitted ahead of proportional
    pace by that factor)."""
    out = []
    ia = ib = 0
    la, lb = len(a), len(b)
    while ia < la or ib < lb:
        if ib >= lb or (ia < la and ia * lb <= ib * la * w):
            out.append(a[ia])
            ia += 1
        else:
            out.append(b[ib])
            ib += 1
    return out


def build(split_waits=True):
    nc = bass.Bass("TRN2", debug=False, num_devices=NCORES)

    # x arrives pre-transposed (feature-major) from the host: [D, NTOK]
    x_d = nc.declare_dram_parameter("x", [D, NTOK], BF, isOutput=False)
    w_d = {}
    b_d = {}
    for nm in ("wq", "wk", "wv", "wo"):
        w_d[nm] = nc.declare_dram_parameter(f"{nm}_w", [D, D], BF, isOutput=False)
        b_d[nm] = nc.declare_dram_parameter(f"{nm}_b", [D], F32, isOutput=False)
    out_d = nc.declare_dram_parameter("out", [NTOK, D], F32, isOutput=True)

    with tile.TileContext(nc) as tc:
        with (
            tc.tile_pool(name="weights", bufs=1) as wpool,
            tc.tile_pool(name="consts", bufs=1) as cpool,
            tc.tile_pool(name="feat", bufs=2) as fpool,
            tc.tile_pool(name="attn", bufs=4) as apool,
            tc.tile_pool(name="outb", bufs=2) as opool,
            tc.tile_pool(name="psum", bufs=2, space="PSUM") as ppool,
        ):
            wt = {nm: [None] * KT for nm in ("wq", "wk", "wv", "wo")}
            biases = {}
            consts = {}
            wtiles = {}

            def unit_load_weight(nm, h=0, halves=1):
                """dma_start for 1/halves of the [D,D] matrix: k-tile k lands
                at cols k*D of a merged [128, KT*D] tile (contiguous 2KB
                runs). Split loads let the first k-tiles' matmuls start while
                the rest of the matrix is still in flight."""

                def f():
                    if nm not in wtiles:
                        wb = wpool.tile(
                            [128, KT * D], BF, tag=f"w_{nm}", name=f"w{nm}"
                        )
                        wtiles[nm] = wb
                        for k in range(KT):
                            wt[nm][k] = wb[:, k * D : (k + 1) * D]
                    wb = wtiles[nm]
                    hk = KT // halves
                    nc.sync.dma_start(
                        out=wb[:, h * hk * D : (h + 1) * hk * D].rearrange(
                            "p (k c) -> p k c", c=D
                        ),
                        in_=w_d[nm][h * hk * 128 : (h + 1) * hk * 128, :].rearrange(
                            "(k p) c -> p k c", p=128
                        ),
                    )

                return f

            def unit_biases_qk():
                def f():
                    # per-partition (feature-major) bias layout for q/k evac
                    for nm in ("wq", "wk"):
                        bt = cpool.tile([128, KT], F32, tag=f"{nm}_pb", name=f"{nm}_pb")
                        nc.sync.dma_start(
                            out=bt[:], in_=b_d[nm][:].rearrange("(m p) -> p m", p=128)
                        )
                        biases[nm] = bt

                return f

            def unit_biases_vo():
                """Broadcast-to-all-partitions bias tiles for v/o via a
                partition-stride-0 DMA read (same row replicated 128x = 1MB
                of queue traffic) -- emitted late so it never delays the
                critical wq/x/wk startup loads."""

                def f():
                    for nm in ("wv", "wo"):
                        bc = cpool.tile([128, D], F32, tag=f"{nm}_bc", name=f"{nm}_bc")
                        nc.sync.dma_start(
                            out=bc[:],
                            in_=b_d[nm][:].unsqueeze(0).broadcast_to((128, D)),
                        )
                        biases[nm] = bc

                return f

            live = {}  # per-chunk tiles handed from stage A to stage B

            def stage_a_units(ch):
                """X load, then QKV projections for chunk ch."""
                tok0 = ch * CHUNK
                st = live.setdefault(ch, {})

                def u_x(h=0, halves=1):
                    """dma_start for 1/halves of the chunk's pre-transposed x:
                    k-tile k lands at cols k*CHUNK of the merged
                    [128, KT*CHUNK] tile (contiguous 1KB runs)."""

                    def f():
                        if "xT" not in st:
                            st["xT"] = fpool.tile(
                                [128, KT * CHUNK], BF, tag="xT", name="xT"
                            )
                        hk = KT // halves
                        nc.sync.dma_start(
                            out=st["xT"][:, h * hk * CHUNK : (h + 1) * hk * CHUNK]
                            .rearrange("p (k t) -> p k t", t=CHUNK),
                            in_=x_d[
                                h * hk * 128 : (h + 1) * hk * 128,
                                tok0 : tok0 + CHUNK,
                            ].rearrange("(k p) t -> p k t", p=128),
                        )

                    return f

                def xT(k):
                    return st["xT"][:, k * CHUNK : (k + 1) * CHUNK]

                def u_q(m):
                    def f():
                        if "qT" not in st:
                            st["qT"] = [
                                fpool.tile([128, CHUNK], BF, tag=f"qT{i}", name=f"qT{i}")
                                for i in range(KT)
                            ]
                        ps = ppool.tile([128, CHUNK], F32, tag="proj", bufs=2, name="ps_q")
                        for k in range(KT):
                            nc.tensor.matmul(
                                ps[:],
                                lhsT=wt["wq"][k][:, m * 128 : (m + 1) * 128],
                                rhs=xT(k),
                                start=(k == 0),
                                stop=(k == KT - 1),
                            )
                        nc.scalar.activation(
                            out=st["qT"][m][:],
                            in_=ps[:],
                            func=mybir.ActivationFunctionType.Identity,
                            bias=biases["wq"][:, m : m + 1],
                        )

                    return f

                def u_k(m):
                    def f():
                        if "kT" not in st:
                            st["kT"] = [
                                fpool.tile([128, CHUNK], BF, tag=f"kT{i}", name=f"kT{i}")
                                for i in range(KT)
                            ]
                        ps = ppool.tile([128, CHUNK], F32, tag="proj", bufs=2, name="ps_k")
                        for k in range(KT):
                            nc.tensor.matmul(
                                ps[:],
                                lhsT=wt["wk"][k][:, m * 128 : (m + 1) * 128],
                                rhs=xT(k),
                                start=(k == 0),
                                stop=(k == KT - 1),
                            )
                        nc.scalar.activation(
                            out=st["kT"][m][:],
                            in_=ps[:],
                            func=mybir.ActivationFunctionType.Identity,
                            bias=biases["wk"][:, m : m + 1],
                        )

                    return f

                def u_v(t, n):
                    def f():
                        if "vaug" not in st:
                            st["vaug"] = [
                                apool.tile(
                                    [128, H * (DH + 1)], BF,
                                    tag=f"vaug{i}", name=f"vaug{i}", bufs=2,
                                )
                                for i in range(TT)
                            ]
                            for i in range(TT):
                                nc.gpsimd.memset(
                                    st["vaug"][i][:]
                                    .rearrange("p (h c) -> p h c", c=DH + 1)[:, :, DH : DH + 1],
                                    1.0,
                                )
                        ps = ppool.tile([128, CHUNK], F32, tag="proj", bufs=2, name="ps_v")
                        for k in range(KT):
                            nc.tensor.matmul(
                                ps[:],
                                lhsT=xT(k)[:, t * 128 : (t + 1) * 128],
                                rhs=wt["wv"][k][:, n * 512 : (n + 1) * 512],
                                start=(k == 0),
                                stop=(k == KT - 1),
                            )
                        nc.vector.tensor_tensor(
                            out=st["vaug"][t][:]
                            .rearrange("p (h c) -> p h c", c=DH + 1)[:, n * 8 : (n + 1) * 8, 0:DH],
                            in0=ps[:].rearrange("p (j c) -> p j c", c=DH),
                            in1=biases["wv"][:, n * 512 : (n + 1) * 512].rearrange(
                                "p (j c) -> p j c", c=DH
                            ),
                            op=mybir.AluOpType.add,
                        )

                    return f

                proj = []
                for m in range(KT):
                    proj.append(u_q(m))
                    proj.append(u_k(m))
                for t in range(TT):
                    for n in range(2):
                        proj.append(u_v(t, n))
                return {
                    "x": [u_x()],
                    "x_quarters": [u_x(qtr, 4) for qtr in range(4)],
                    "q": [u_q(m) for m in range(KT)],
                    "k": [u_k(m) for m in range(KT)],
                    "v": [u_v(t, n) for t in range(TT) for n in range(2)],
                    "proj": proj,
                }

            def attn_core_units(ch, with_out=False):
                """Attention for chunk ch, software-pipelined per token-tile:
                the score pair (t+1) is emitted before ctx(t) so the ACT exp
                latency hides behind the next pair's score matmuls."""
                st = live[ch]
                es_tiles = {}

                def u_scores(u, t):
                    def f():
                        qT, kT = st["qT"], st["kT"]
                        es = apool.tile([128, 128], BF, tag="expS", name="es")
                        es_tiles[(u, t)] = es
                        for hh in (0, 1):
                            hsl = slice(hh * 64, hh * 64 + 64)
                            ps_s = ppool.tile([128, 64], F32, tag="sc", bufs=4, name=f"ps_s{hh}")
                            for bpar in (0, 1):
                                toksl = slice(u * 128 + bpar * 64, u * 128 + bpar * 64 + 64)
                                nc.tensor.matmul(
                                    ps_s[bpar * 64 : bpar * 64 + 64, :],
                                    lhsT=kT[t][hsl, toksl],
                                    rhs=qT[t][hsl, toksl],
                                    start=True,
                                    stop=True,
                                )
                            nc.scalar.activation(
                                out=es[:, hsl],
                                in_=ps_s[:],
                                func=mybir.ActivationFunctionType.Exp,
                                scale=float(SCALE),
                            )

                    return f

                def u_ctx(u, t):
                    def f():
                        if "ctx" not in st:
                            st["ctx"] = [
                                apool.tile([128, D], BF, tag=f"ctx{i}", name=f"ctx{i}", bufs=2)
                                for i in range(TT)
                            ]
                        vaug, ctx = st["vaug"], st["ctx"]
                        es = es_tiles.pop((u, t))
                        ps_c = ppool.tile([128, 130], F32, tag="cx", bufs=2, name="ps_c")
                        for bpar in (0, 1):
                            bsl = slice(bpar * 64, bpar * 64 + 64)
                            for hh in (0, 1):
                                h = 2 * t + hh
                                nc.tensor.matmul(
                                    ps_c[bsl, hh * 65 : hh * 65 + 65],
                                    lhsT=es[bsl, hh * 64 : hh * 64 + 64],
                                    rhs=vaug[u][bsl, h * 65 : (h + 1) * 65],
                                    start=True,
                                    stop=True,
                                )
                        for hh in (0, 1):
                            h = 2 * t + hh
                            rc = apool.tile([128, 1], F32, tag="recip", name="rc")
                            nc.vector.reciprocal(
                                rc[:], ps_c[:, hh * 65 + DH : hh * 65 + DH + 1]
                            )
                            nc.vector.tensor_scalar(
                                out=ctx[u][:, h * DH : (h + 1) * DH],
                                in0=ps_c[:, hh * 65 : hh * 65 + DH],
                                scalar1=rc[:],
                                scalar2=None,
                                op0=mybir.AluOpType.mult,
                            )

                    return f

                def u_ctxT(u):
                    """Transpose ctx[u] [128 toks, 1024 feats] into the cT
                    blocks with ONE xbar DMA-transpose (off the PE): the 3D
                    dst AP [128, k, 128] receives transposed row k*128+pp at
                    (partition pp, block k) -- verified on hardware."""

                    def f():
                        if "cT" not in st:
                            st["cT"] = fpool.tile(
                                [128, KT * CHUNK], BF, tag="cT", name="cT", bufs=3
                            )
                        cT_blocks = st["cT"].rearrange("p (k c) -> p k c", c=CHUNK)
                        nc.sync.dma_start_transpose(
                            out=cT_blocks[:, :, u * 128 : (u + 1) * 128],
                            in_=st["ctx"][u][:],
                        )

                    return f

                units = []
                ou = out_units(ch) if with_out else None
                for u in range(TT):
                    units.append(u_scores(u, 0))
                    for t in range(KT - 1):
                        units.append(u_scores(u, t + 1))
                        units.append(u_ctx(u, t))
                    units.append(u_ctx(u, KT - 1))
                    units.append(u_ctxT(u))
                    if ou is not None:
                        # this token-tile's output projection can start as
                        # soon as its ctxT landed — keeps the tail dense
                        units.extend(ou[u * 2 : u * 2 + 2])
                return units

            def out_units(ch):
                """Output projection + gelu + store for chunk ch (needs cT)."""
                tok0 = ch * CHUNK
                st = live[ch]

                def u_out(t, n):
                    def f():
                        cT = st["cT"]
                        ps = ppool.tile([128, CHUNK], F32, tag="proj", bufs=2, name="ps_o")
                        for k in range(KT):
                            nc.tensor.matmul(
                                ps[:],
                                lhsT=cT[:, k * CHUNK + t * 128 : k * CHUNK + (t + 1) * 128],
                                rhs=wt["wo"][k][:, n * 512 : (n + 1) * 512],
                                start=(k == 0),
                                stop=(k == KT - 1),
                            )
                        tmp = opool.tile([128, 512], F32, tag="obuf", name="tmp")
                        nc.vector.tensor_tensor(
                            out=tmp[:],
                            in0=ps[:],
                            in1=biases["wo"][:, n * 512 : (n + 1) * 512],
                            op=mybir.AluOpType.add,
                        )
                        og = opool.tile([128, 512], F32, tag="ogelu", name="og")
                        nc.scalar.activation(
                            out=og[:], in_=tmp[:], func=mybir.ActivationFunctionType.Gelu
                        )
                        nc.sync.dma_start(
                            out=out_d[
                                tok0 + t * 128 : tok0 + (t + 1) * 128,
                                n * 512 : (n + 1) * 512,
                            ],
                            in_=og[:],
                        )

                    return f

                return [u_out(t, n) for t in range(TT) for n in range(2)]

            # ---- emission ----
            stages = [stage_a_units(ch) for ch in range(NCH)]
            # prologue: first halves of wq + x(0) land first so the Q
            # projection's k=0..3 matmuls can start while the second halves
            # are still in flight; then chunk-0 projections interleaved with
            # the remaining weight loads and x(1)
            for qtr in range(4):
                unit_load_weight("wq", qtr, 4)()
                stages[0]["x_quarters"][qtr]()
            unit_biases_qk()()
            unit_load_weight("wk")()
            for u in _interleave(stages[0]["q"], stages[1]["x"]):
                u()
            unit_load_weight("wv")()
            unit_biases_vo()()
            for u in stages[0]["k"]:
                u()
            unit_load_weight("wo")()
            for u in stages[0]["v"]:
                u()
            # steady state: window ch emits proj(ch) + x(ch+1) + attention
            # core of ch-1 + output projection of ch-2 (the delay keeps the
            # tail full of dense work to interleave with the final attention)
            for ch in range(1, NCH):
                dense = stages[ch]["proj"]
                if ch + 1 < NCH:
                    dense = _interleave(dense, stages[ch + 1]["x"])
                if ch - 3 >= 0:
                    dense = _interleave(dense, out_units(ch - 3))
                for u in _interleave(dense, attn_core_units(ch - 1)):
                    u()
                if ch - 3 >= 0:
                    live.pop(ch - 3)
            # tail: last chunk's attention (with its own output projection
            # inlined per token-tile) over the pending chunk's output
            # projection.
            pend = out_units(NCH - 3) + out_units(NCH - 2)
            for u in _interleave(attn_core_units(NCH - 1, with_out=True), pend):
                u()
            live.pop(NCH - 3)
            live.pop(NCH - 2)
            live.pop(NCH - 1)

    if split_waits:
        _split_multiwait(nc)
    return nc


_NC = None


def _get_nc():
    global _NC
    if _NC is None:
        _NC = build()
    return _NC


def _make_in_maps(inputs):
    x = np.asarray(inputs["x"], dtype=np.float32).astype(ml_dtypes.bfloat16)
    full = {}
    for nm in ("wq", "wk", "wv", "wo"):
        full[f"{nm}_w"] = np.ascontiguousarray(
            np.asarray(inputs[f"{nm}_w"], dtype=np.float32).astype(ml_dtypes.bfloat16)
        )
        full[f"{nm}_b"] = np.ascontiguousarray(
            np.asarray(inputs[f"{nm}_b"], dtype=np.float32)
        )
    in_maps = []
    for c in range(NCORES):
        # pre-transpose to feature-major [D, NTOK] so no on-chip x transpose
        xc = np.ascontiguousarray(x[c * BL : (c + 1) * BL].reshape(NTOK, D).T)
        m = {"x": xc}
        m.update(full)
        in_maps.append(m)
    return in_maps


def kernel(**inputs):
    nc = _get_nc()
    res = run_bass_kernel_spmd(
        nc, _make_in_maps(inputs), core_ids=list(range(NCORES))
    ).results
    parts = [res[c]["out"].reshape(BL, 8, 8, D) for c in range(NCORES)]
    return np.concatenate(parts, axis=0)


def kernel_profiled(**inputs):
    """Like kernel() but requests an NTFF trace; returns (out, exec_time_ns, raw)."""
    nc = _get_nc()
    r = run_bass_kernel_spmd(
        nc, _make_in_maps(inputs), core_ids=list(range(NCORES)), trace=True
    )
    parts = [r.results[c]["out"].reshape(BL, 8, 8, D) for c in range(NCORES)]
    return np.concatenate(parts, axis=0), r.exec_time_ns, r



# revision 5
# speedup vs baseline: 1.1751x; 1.1751x over previous
"""Multi-head attention (B=512,S=64,D=1024,H=16) on 8 trn2 NeuronCores.

Strategy: pure data-parallel over the batch dim — each core gets 64 batches
(4096 tokens) and runs the full fused MHA layer locally; no collectives.

Per-core dataflow (token chunks of 512 = 8 batches):
  xT [1024, tok] arrives pre-transposed (feature-major) from the host in
  TWO dtypes: fp8(e4m3) for the Q/K projections and bf16 for V.
  qT = Wq.T @ xT, kT = Wk.T @ xT  via fp8 DoubleRow matmuls (2 k-tiles per
  instruction, 2x bf16 throughput; weights host-scaled by 1024 to dodge
  e4m3 denormals, descaled in the ACT evac via scale=1/1024). Accuracy:
  fp8 Q/K only shifts logits ~3% pre-softmax -> rel err 7e-3 (budget 2e-2);
  fp8 V or O would push the v-path error to 4e-2, so those stay bf16.
  v  = x @ Wv                        (token-major, interleaved with ones col)
  scoresT[k,q] = (kT slice).T @ (qT slice)   per (batch,head), quadrant packed
  expS = exp(scoresT/32)             (no max-subtract: logits are ~N(0,0.1))
  ctx[q,:]|sumexp[q] = expS.T @ [v|1]        -> normalize with per-partition
  recip (both heads' reciprocals in one DVE op); ctxT via PE transposes
  batched on the xbar DMA engine; out = gelu(ctx @ Wo) token-major -> DRAM.

The ACT engine reloads its function table (~1.3us) on every exp<->gelu
switch; gelus are therefore split out of the out-projection units and
emitted as one consecutive batch per chunk (2 table loads/chunk instead
of ~14). The out unit keeps matmul + DVE bias-add inline (tmp tiles,
bufs=8) and the gelu+store batch drains them at the window boundary.

Weights/x are converted on the host and loaded with one merged dma_start
per matrix (quarters for wq/x8(0) so the first projection matmuls start
while the rest is in flight). The PE has a HAM power budget (~536us of
full-speed execution, then 4/8 duty-cycling); fp8 keeps PE busy well
under it so the whole run stays at full clock.

PSUM packing rule (hardware): two concurrent matmuls may share a PSUM bank
only if they use the same array row-strip (same operand base partition) or
a strict diagonal (row,col) placement; different row-strips draining into
one bank is fatal. All sharing here uses full-128 row strips.
PSUM budget (bank-granular): proj 2 + sc 4 + cx 2 = 8 banks.
"""

import sys

sys.path.insert(0, "/opt/trn_rl_repo")

import numpy as np
import ml_dtypes

import concourse.bass as bass
import concourse.tile as tile
from concourse import mybir
from concourse.bass_utils import run_bass_kernel_spmd

F32 = mybir.dt.float32
BF = mybir.dt.bfloat16
F8 = mybir.dt.float8e4

B, S, D, H = 512, 64, 1024, 16
DH = D // H  # 64
NCORES = 8
BL = B // NCORES  # 64 batches per core
NTOK = BL * S  # 4096 tokens per core
CHUNK = 512  # tokens per pipeline chunk (8 batches)
NCH = NTOK // CHUNK  # 8
TT = CHUNK // 128  # 4 token-tiles per chunk
KT = D // 128  # 8 d-tiles
SCALE = 1.0 / np.sqrt(np.float32(D))  # 1/32
WSCALE = 1024.0  # host-side premultiplier for the fp8 wq/wk weights


def _split_multiwait(nc, limit=1):
    """walrus can emit at most one sync-wait per instruction; TileContext's
    tail drain carries one wait per touched processor. Hoist extras onto
    chained NOPs."""
    f = nc.m.functions[0]
    for blk in f.blocks:
        new_insts = []
        for inst in blk.instructions:
            si = inst.sync_info
            if si is not None and len(si.on_wait) > limit:
                extra = si.on_wait[:-limit]
                keep = si.on_wait[-limit:]
                for i, w in enumerate(extra):
                    nop = mybir.InstNoOp(
                        name=f"{inst.name}-waitsplit{i}",
                        sync_info=mybir.SyncInfo(on_wait=[w], on_update=[]),
                        bass_nofuse=True,
                        ins=[],
                        outs=[],
                    )
                    nop.engine = inst.engine
                    new_insts.append(nop)
                si.on_wait[:] = keep
            new_insts.append(inst)
        blk.instructions[:] = new_insts


def _interleave(a, b, w=1.0):
    """Merge two unit lists round-robin, proportionally to their lengths.
    w > 1 front-loads list a (a's units are emitted ahead of proportional
    pace by that factor)."""
    out = []
    ia = ib = 0
    la, lb = len(a), len(b)
    while ia < la or ib < lb:
        if ib >= lb or (ia < la and ia * lb <= ib * la * w):
            out.append(a[ia])
            ia += 1
        else:
            out.append(b[ib])
            ib += 1
    return out


def build(split_waits=True):
    nc = bass.Bass("TRN2", debug=False, num_devices=NCORES)

    # x arrives pre-transposed (feature-major): bf16 for V, e4m3 for Q/K
    x_d = nc.declare_dram_parameter("x", [D, NTOK], BF, isOutput=False)
    x8_d = nc.declare_dram_parameter("x8", [D, NTOK], F8, isOutput=False)
    w_d = {}
    b_d = {}
    for nm, dt in (("wq", F8), ("wk", F8), ("wv", BF), ("wo", BF)):
        w_d[nm] = nc.declare_dram_parameter(f"{nm}_w", [D, D], dt, isOutput=False)
        b_d[nm] = nc.declare_dram_parameter(f"{nm}_b", [D], F32, isOutput=False)
    out_d = nc.declare_dram_parameter("out", [NTOK, D], F32, isOutput=True)

    with tile.TileContext(nc) as tc:
        with (
            tc.tile_pool(name="weights", bufs=1) as wpool,
            tc.tile_pool(name="consts", bufs=1) as cpool,
            tc.tile_pool(name="feat", bufs=2) as fpool,
            tc.tile_pool(name="attn", bufs=4) as apool,
            tc.tile_pool(name="outb", bufs=2) as opool,
            tc.tile_pool(name="psum", bufs=2, space="PSUM") as ppool,
        ):
            wt = {nm: [None] * KT for nm in ("wv", "wo")}
            biases = {}
            wtiles = {}
            WDT = {"wq": F8, "wk": F8, "wv": BF, "wo": BF}

            def unit_load_weight(nm, h=0, halves=1):
                """dma_start for 1/halves of the [D,D] matrix: k-tile k lands
                at cols k*D of a merged [128, KT*D] tile (contiguous runs).
                Split loads let the first k-tiles' matmuls start while the
                rest of the matrix is still in flight."""

                def f():
                    if nm not in wtiles:
                        wb = wpool.tile(
                            [128, KT * D], WDT[nm], tag=f"w_{nm}", name=f"w{nm}"
                        )
                        wtiles[nm] = wb
                        if nm in wt:
                            for k in range(KT):
                                wt[nm][k] = wb[:, k * D : (k + 1) * D]
                    wb = wtiles[nm]
                    hk = KT // halves
                    nc.sync.dma_start(
                        out=wb[:, h * hk * D : (h + 1) * hk * D].rearrange(
                            "p (k c) -> p k c", c=D
                        ),
                        in_=w_d[nm][h * hk * 128 : (h + 1) * hk * 128, :].rearrange(
                            "(k p) c -> p k c", p=128
                        ),
                    )

                return f

            def wpair(nm, kj, m):
                """[128, 2, 128] lhsT AP for DoubleRow: k-tiles (2kj, 2kj+1),
                output-feature block m."""
                wb = wtiles[nm]
                return wb[:, 2 * kj * D : (2 * kj + 2) * D].rearrange(
                    "p (k c) -> p k c", c=D
                )[:, :, m * 128 : (m + 1) * 128]

            def unit_biases_qk():
                def f():
                    # per-partition (feature-major) bias layout for q/k evac
                    for nm in ("wq", "wk"):
                        bt = cpool.tile([128, KT], F32, tag=f"{nm}_pb", name=f"{nm}_pb")
                        nc.sync.dma_start(
                            out=bt[:], in_=b_d[nm][:].rearrange("(m p) -> p m", p=128)
                        )
                        biases[nm] = bt

                return f

            def unit_biases_vo():
                """Broadcast-to-all-partitions bias tiles for v/o via a
                partition-stride-0 DMA read -- emitted late so it never
                delays the critical wq/x8/wk startup loads."""

                def f():
                    for nm in ("wv", "wo"):
                        bc = cpool.tile([128, D], F32, tag=f"{nm}_bc", name=f"{nm}_bc")
                        nc.sync.dma_start(
                            out=bc[:],
                            in_=b_d[nm][:].unsqueeze(0).broadcast_to((128, D)),
                        )
                        biases[nm] = bc

                return f

            live = {}  # per-chunk tiles handed from stage A to stage B

            def stage_a_units(ch):
                """X loads, then QKV projections for chunk ch."""
                tok0 = ch * CHUNK
                st = live.setdefault(ch, {})

                def u_x(h=0, halves=1):
                    """bf16 x chunk load (V projection operand)."""

                    def f():
                        if "xT" not in st:
                            st["xT"] = fpool.tile(
                                [128, KT * CHUNK], BF, tag="xT", name="xT"
                            )
                        hk = KT // halves
                        nc.sync.dma_start(
                            out=st["xT"][:, h * hk * CHUNK : (h + 1) * hk * CHUNK]
                            .rearrange("p (k t) -> p k t", t=CHUNK),
                            in_=x_d[
                                h * hk * 128 : (h + 1) * hk * 128,
                                tok0 : tok0 + CHUNK,
                            ].rearrange("(k p) t -> p k t", p=128),
                        )

                    return f

                def u_x8(h=0, halves=1):
                    """fp8 x chunk load (Q/K projection operand)."""

                    def f():
                        if "x8T" not in st:
                            st["x8T"] = fpool.tile(
                                [128, KT * CHUNK], F8, tag="x8T", name="x8T"
                            )
                        hk = KT // halves
                        nc.sync.dma_start(
                            out=st["x8T"][:, h * hk * CHUNK : (h + 1) * hk * CHUNK]
                            .rearrange("p (k t) -> p k t", t=CHUNK),
                            in_=x8_d[
                                h * hk * 128 : (h + 1) * hk * 128,
                                tok0 : tok0 + CHUNK,
                            ].rearrange("(k p) t -> p k t", p=128),
                        )

                    return f

                def xT(k):
                    return st["xT"][:, k * CHUNK : (k + 1) * CHUNK]

                def x8pair(kj):
                    return st["x8T"][
                        :, 2 * kj * CHUNK : (2 * kj + 2) * CHUNK
                    ].rearrange("p (k t) -> p k t", t=CHUNK)

                def u_qk(nm, dst, m):
                    def f():
                        if dst not in st:
                            st[dst] = [
                                fpool.tile(
                                    [128, CHUNK], BF, tag=f"{dst}{i}", name=f"{dst}{i}"
                                )
                                for i in range(KT)
                            ]
                        ps = ppool.tile(
                            [128, CHUNK], F32, tag="proj", bufs=2, name=f"ps_{dst}"
                        )
                        for kj in range(KT // 2):
                            nc.tensor.matmul(
                                ps[:],
                                lhsT=wpair(nm, kj, m),
                                rhs=x8pair(kj),
                                start=(kj == 0),
                                stop=(kj == KT // 2 - 1),
                                perf_mode=mybir.MatmulPerfMode.DoubleRow,
                            )
                        nc.scalar.activation(
                            out=st[dst][m][:],
                            in_=ps[:],
                            func=mybir.ActivationFunctionType.Identity,
                            bias=biases[nm][:, m : m + 1],
                            scale=1.0 / WSCALE,
                        )

                    return f

                def u_v(t, n):
                    def f():
                        if "vaug" not in st:
                            st["vaug"] = [
                                apool.tile(
                                    [128, H * (DH + 1)], BF,
                                    tag=f"vaug{i}", name=f"vaug{i}", bufs=2,
                                )
                                for i in range(TT)
                            ]
                            for i in range(TT):
                                nc.gpsimd.memset(
                                    st["vaug"][i][:]
                                    .rearrange("p (h c) -> p h c", c=DH + 1)[:, :, DH : DH + 1],
                                    1.0,
                                )
                        ps = ppool.tile([128, CHUNK], F32, tag="proj", bufs=2, name="ps_v")
                        for k in range(KT):
                            nc.tensor.matmul(
                                ps[:],
                                lhsT=xT(k)[:, t * 128 : (t + 1) * 128],
                                rhs=wt["wv"][k][:, n * 512 : (n + 1) * 512],
                                start=(k == 0),
                                stop=(k == KT - 1),
                            )
                        nc.vector.tensor_tensor(
                            out=st["vaug"][t][:]
                            .rearrange("p (h c) -> p h c", c=DH + 1)[:, n * 8 : (n + 1) * 8, 0:DH],
                            in0=ps[:].rearrange("p (j c) -> p j c", c=DH),
                            in1=biases["wv"][:, n * 512 : (n + 1) * 512].rearrange(
                                "p (j c) -> p j c", c=DH
                            ),
                            op=mybir.AluOpType.add,
                        )

                    return f

                proj = []
                for m in range(KT):
                    proj.append(u_qk("wq", "qT", m))
                    proj.append(u_qk("wk", "kT", m))
                for t in range(TT):
                    for n in range(2):
                        proj.append(u_v(t, n))
                return {
                    "x": [u_x8(), u_x()],
                    "x8_quarters": [u_x8(qtr, 4) for qtr in range(4)],
                    "x_bf": [u_x()],
                    "q": [u_qk("wq", "qT", m) for m in range(KT)],
                    "k": [u_qk("wk", "kT", m) for m in range(KT)],
                    "v": [u_v(t, n) for t in range(TT) for n in range(2)],
                    "proj": proj,
                }

            def attn_core_units(ch, with_out=False):
                """Attention for chunk ch, software-pipelined per token-tile:
                the score pair (t+1) is emitted before ctx(t) so the ACT exp
                latency hides behind the next pair's score matmuls."""
                st = live[ch]
                es_tiles = {}

                def u_scores(u, t):
                    def f():
                        qT, kT = st["qT"], st["kT"]
                        es = apool.tile([128, 128], BF, tag="expS", name="es")
                        es_tiles[(u, t)] = es
                        for hh in (0, 1):
                            hsl = slice(hh * 64, hh * 64 + 64)
                            ps_s = ppool.tile([128, 64], F32, tag="sc", bufs=4, name=f"ps_s{hh}")
                            for bpar in (0, 1):
                                toksl = slice(u * 128 + bpar * 64, u * 128 + bpar * 64 + 64)
                                nc.tensor.matmul(
                                    ps_s[bpar * 64 : bpar * 64 + 64, :],
                                    lhsT=kT[t][hsl, toksl],
                                    rhs=qT[t][hsl, toksl],
                                    start=True,
                                    stop=True,
                                )
                            nc.scalar.activation(
                                out=es[:, hsl],
                                in_=ps_s[:],
                                func=mybir.ActivationFunctionType.Exp,
                                scale=float(SCALE),
                            )

                    return f

                def u_ctx(u, t):
                    def f():
                        if "ctx" not in st:
                            st["ctx"] = [
                                apool.tile([128, D], BF, tag=f"ctx{i}", name=f"ctx{i}", bufs=2)
                                for i in range(TT)
                            ]
                        vaug, ctx = st["vaug"], st["ctx"]
                        es = es_tiles.pop((u, t))
                        ps_c = ppool.tile([128, 130], F32, tag="cx", bufs=2, name="ps_c")
                        for bpar in (0, 1):
                            bsl = slice(bpar * 64, bpar * 64 + 64)
                            for hh in (0, 1):
                                h = 2 * t + hh
                                nc.tensor.matmul(
                                    ps_c[bsl, hh * 65 : hh * 65 + 65],
                                    lhsT=es[bsl, hh * 64 : hh * 64 + 64],
                                    rhs=vaug[u][bsl, h * 65 : (h + 1) * 65],
                                    start=True,
                                    stop=True,
                                )
                        # both heads' sumexp slots (cols 64, 129) -> one recip
                        rc = apool.tile([128, 2], F32, tag="recip", name="rc")
                        nc.vector.reciprocal(
                            rc[:].rearrange("p (h c) -> p h c", c=1),
                            ps_c[:].rearrange("p (h c) -> p h c", c=65)[:, :, DH : DH + 1],
                        )
                        for hh in (0, 1):
                            h = 2 * t + hh
                            nc.vector.tensor_scalar(
                                out=ctx[u][:, h * DH : (h + 1) * DH],
                                in0=ps_c[:, hh * 65 : hh * 65 + DH],
                                scalar1=rc[:, hh : hh + 1],
                                scalar2=None,
                                op0=mybir.AluOpType.mult,
                            )

                    return f

                def u_ctxT(u):
                    """Transpose ctx[u] [128 toks, 1024 feats] into the cT
                    blocks with ONE xbar DMA-transpose (off the PE): the 3D
                    dst AP [128, k, 128] receives transposed row k*128+pp at
                    (partition pp, block k) -- verified on hardware."""

                    def f():
                        if "cT" not in st:
                            st["cT"] = fpool.tile(
                                [128, KT * CHUNK], BF, tag="cT", name="cT", bufs=3
                            )
                        cT_blocks = st["cT"].rearrange("p (k c) -> p k c", c=CHUNK)
                        nc.sync.dma_start_transpose(
                            out=cT_blocks[:, :, u * 128 : (u + 1) * 128],
                            in_=st["ctx"][u][:],
                        )

                    return f

                groups = []
                for u in range(TT):
                    units = [u_scores(u, 0)]
                    for t in range(KT - 1):
                        units.append(u_scores(u, t + 1))
                        units.append(u_ctx(u, t))
                    units.append(u_ctx(u, KT - 1))
                    units.append(u_ctxT(u))
                    groups.append(units)
                if with_out:
                    return groups
                return [u for g in groups for u in g]

            def out_mm_units(ch):
                """Output-projection matmul + DVE bias-add into a tmp tile
                (gelu+store split out so gelus can batch on ACT)."""
                st = live[ch]

                def u_mm(t, n):
                    def f():
                        cT = st["cT"]
                        ps = ppool.tile([128, CHUNK], F32, tag="proj", bufs=2, name="ps_o")
                        for k in range(KT):
                            nc.tensor.matmul(
                                ps[:],
                                lhsT=cT[:, k * CHUNK + t * 128 : k * CHUNK + (t + 1) * 128],
                                rhs=wt["wo"][k][:, n * 512 : (n + 1) * 512],
                                start=(k == 0),
                                stop=(k == KT - 1),
                            )
                        tmp = opool.tile([128, 512], F32, tag="obuf", bufs=8, name="tmp")
                        st.setdefault("otmp", {})[(t, n)] = tmp
                        nc.vector.tensor_tensor(
                            out=tmp[:],
                            in0=ps[:],
                            in1=biases["wo"][:, n * 512 : (n + 1) * 512],
                            op=mybir.AluOpType.add,
                        )

                    return f

                return [u_mm(t, n) for t in range(TT) for n in range(2)]

            def out_act_units(ch):
                """Gelu + DRAM store for chunk ch (emitted as one batch per
                chunk: ACT reloads its function table on exp<->gelu switches)."""
                tok0 = ch * CHUNK
                st = live[ch]

                def u_act(t, n):
                    def f():
                        tmp = st["otmp"].pop((t, n))
                        og = opool.tile([128, 512], F32, tag="ogelu", name="og")
                        nc.scalar.activation(
                            out=og[:], in_=tmp[:], func=mybir.ActivationFunctionType.Gelu
                        )
                        nc.sync.dma_start(
                            out=out_d[
                                tok0 + t * 128 : tok0 + (t + 1) * 128,
                                n * 512 : (n + 1) * 512,
                            ],
                            in_=og[:],
                        )

                    return f

                return [u_act(t, n) for t in range(TT) for n in range(2)]

            # ---- emission ----
            stages = [stage_a_units(ch) for ch in range(NCH)]
            # prologue: first quarters of wq + x8(0) land first so the Q
            # projection's first DoubleRow matmuls can start while the rest
            # is still in flight; then chunk-0 projections interleaved with
            # the remaining weight loads and x(1)
            for qtr in range(4):
                unit_load_weight("wq", qtr, 4)()
                stages[0]["x8_quarters"][qtr]()
            unit_biases_qk()()
            unit_load_weight("wk")()
            for u in _interleave(stages[0]["q"], stages[0]["x_bf"] + stages[1]["x"]):
                u()
            unit_load_weight("wv")()
            unit_biases_vo()()
            for u in stages[0]["k"]:
                u()
            unit_load_weight("wo")()
            for u in stages[0]["v"]:
                u()
            # steady state: window ch emits proj(ch) + x(ch+1) + attention
            # core of ch-1 + output projection of ch-3 (the delay keeps the
            # tail full of dense work to interleave with the final attention)
            for ch in range(1, NCH):
                dense = stages[ch]["proj"]
                if ch + 1 < NCH:
                    dense = _interleave(dense, stages[ch + 1]["x"])
                if ch - 3 >= 0:
                    dense = _interleave(dense, out_mm_units(ch - 3))
                for u in _interleave(dense, attn_core_units(ch - 1)):
                    u()
                if ch - 3 >= 0:
                    # batched gelu+store drain for chunk ch-3 (2 ACT table
                    # loads per chunk instead of one per out unit)
                    for u in out_act_units(ch - 3):
                        u()
                    live.pop(ch - 3)
            # tail: the last chunk's attention, split into token-tile groups,
            # carries the three pending output projections one chunk at a
            # time — each chunk's gelu batch drains its 8 obuf tmp tiles
            # before the next chunk's bias-adds allocate them (the obuf ring
            # holds only 8; overlapping two chunks' allocations deadlocks
            # against the later-emitted gelus).
            groups = attn_core_units(NCH - 1, with_out=True)
            last_mm = out_mm_units(NCH - 1)
            # mm(7, t) reads cT block t — it must be EMITTED after ctxT(t)
            # or Tile orders the write after the read and it consumes stale
            # ring-buffer data; only t0..t2 may interleave with group 3.
            for gs, mm, act_ch in (
                (groups[0] + groups[1], out_mm_units(NCH - 3), NCH - 3),
                (groups[2], out_mm_units(NCH - 2), NCH - 2),
                (groups[3], last_mm[:6], None),
            ):
                for u in _interleave(gs, mm):
                    u()
                if act_ch is not None:
                    for u in out_act_units(act_ch):
                        u()
            for u in last_mm[6:]:
                u()
            for u in out_act_units(NCH - 1):
                u()
            live.pop(NCH - 3)
            live.pop(NCH - 2)
            live.pop(NCH - 1)

    if split_waits:
        _split_multiwait(nc)
    return nc


_NC = None


def _get_nc():
    global _NC
    if _NC is None:
        _NC = build()
    return _NC


def _make_in_maps(inputs):
    xf = np.asarray(inputs["x"], dtype=np.float32)
    full = {}
    for nm in ("wq", "wk", "wv", "wo"):
        wf = np.asarray(inputs[f"{nm}_w"], dtype=np.float32)
        if nm in ("wq", "wk"):
            # fp8 weights, pre-scaled by 1024 to stay in e4m3 normal range
            full[f"{nm}_w"] = np.ascontiguousarray(
                np.clip(wf * WSCALE, -240.0, 240.0).astype(ml_dtypes.float8_e4m3)
            )
        else:
            full[f"{nm}_w"] = np.ascontiguousarray(wf.astype(ml_dtypes.bfloat16))
        full[f"{nm}_b"] = np.ascontiguousarray(
            np.asarray(inputs[f"{nm}_b"], dtype=np.float32)
        )
    in_maps = []
    for c in range(NCORES):
        # pre-transpose to feature-major [D, NTOK] so no on-chip x transpose
        xc = np.ascontiguousarray(xf[c * BL : (c + 1) * BL].reshape(NTOK, D).T)
        m = {
            "x": xc.astype(ml_dtypes.bfloat16),
            "x8": np.clip(xc, -240.0, 240.0).astype(ml_dtypes.float8_e4m3),
        }
        m.update(full)
        in_maps.append(m)
    return in_maps


def kernel(**inputs):
    nc = _get_nc()
    res = run_bass_kernel_spmd(
        nc, _make_in_maps(inputs), core_ids=list(range(NCORES))
    ).results
    parts = [res[c]["out"].reshape(BL, 8, 8, D) for c in range(NCORES)]
    return np.concatenate(parts, axis=0)


def kernel_profiled(**inputs):
    """Like kernel() but requests an NTFF trace; returns (out, exec_time_ns, raw)."""
    nc = _get_nc()
    r = run_bass_kernel_spmd(
        nc, _make_in_maps(inputs), core_ids=list(range(NCORES)), trace=True
    )
    parts = [r.results[c]["out"].reshape(BL, 8, 8, D) for c in range(NCORES)]
    return np.concatenate(parts, axis=0), r.exec_time_ns, r


# revision 6
# speedup vs baseline: 1.1950x; 1.0169x over previous
"""Multi-head attention (B=512,S=64,D=1024,H=16) on 8 trn2 NeuronCores.

Strategy: pure data-parallel over the batch dim — each core gets 64 batches
(4096 tokens) and runs the full fused MHA layer locally; no collectives.

Per-core dataflow (token chunks of 512 = 8 batches):
  xT [1024, tok] arrives pre-transposed (feature-major) from the host in
  TWO dtypes: fp8(e4m3) for the Q/K projections and bf16 for V.
  qT = Wq.T @ xT, kT = Wk.T @ xT  via fp8 DoubleRow matmuls (2 k-tiles per
  instruction, 2x bf16 throughput; weights host-scaled by 1024 to dodge
  e4m3 denormals, descaled in the ACT evac via scale=1/1024). Accuracy:
  fp8 Q/K only shifts logits ~3% pre-softmax -> rel err 7e-3 (budget 2e-2);
  fp8 V or O would push the v-path error to 4e-2, so those stay bf16.
  v  = x @ Wv                        (token-major, interleaved with ones col)
  scoresT[k,q] = (kT slice).T @ (qT slice)   per (batch,head), quadrant packed
  expS = exp(scoresT/32)             (no max-subtract: logits are ~N(0,0.1))
  ctx[q,:]|sumexp[q] = expS.T @ [v|1]        -> normalize with per-partition
  recip (both heads' reciprocals in one DVE op); ctxT via PE transposes
  batched on the xbar DMA engine; out = gelu(ctx @ Wo) token-major -> DRAM.

The ACT engine reloads its function table (~1.3us) on every exp<->gelu
switch; gelus are therefore split out of the out-projection units and
emitted as one consecutive batch per chunk (2 table loads/chunk instead
of ~14). The out unit keeps matmul + DVE bias-add inline (tmp tiles,
bufs=8) and the gelu+store batch drains them at the window boundary.

Weights/x are converted on the host and loaded with one merged dma_start
per matrix (quarters for wq/x8(0) so the first projection matmuls start
while the rest is in flight). The PE has a HAM power budget (~536us of
full-speed execution, then 4/8 duty-cycling); fp8 keeps PE busy well
under it so the whole run stays at full clock.

PSUM packing rule (hardware): two concurrent matmuls may share a PSUM bank
only if they use the same array row-strip (same operand base partition) or
a strict diagonal (row,col) placement; different row-strips draining into
one bank is fatal. All sharing here uses full-128 row strips.
PSUM budget (bank-granular): proj 2 + sc 4 + cx 2 = 8 banks.
"""

import sys

sys.path.insert(0, "/opt/trn_rl_repo")

import numpy as np
import ml_dtypes

import concourse.bass as bass
import concourse.tile as tile
from concourse import mybir
from concourse.bass_utils import run_bass_kernel_spmd

F32 = mybir.dt.float32
BF = mybir.dt.bfloat16
F8 = mybir.dt.float8e4

B, S, D, H = 512, 64, 1024, 16
DH = D // H  # 64
NCORES = 8
BL = B // NCORES  # 64 batches per core
NTOK = BL * S  # 4096 tokens per core
CHUNK = 512  # tokens per pipeline chunk (8 batches)
NCH = NTOK // CHUNK  # 8
TT = CHUNK // 128  # 4 token-tiles per chunk
KT = D // 128  # 8 d-tiles
SCALE = 1.0 / np.sqrt(np.float32(D))  # 1/32
WSCALE = 1024.0  # host-side premultiplier for the fp8 wq/wk weights


def _split_multiwait(nc, limit=1):
    """walrus can emit at most one sync-wait per instruction; TileContext's
    tail drain carries one wait per touched processor. Hoist extras onto
    chained NOPs."""
    f = nc.m.functions[0]
    for blk in f.blocks:
        new_insts = []
        for inst in blk.instructions:
            si = inst.sync_info
            if si is not None and len(si.on_wait) > limit:
                extra = si.on_wait[:-limit]
                keep = si.on_wait[-limit:]
                for i, w in enumerate(extra):
                    nop = mybir.InstNoOp(
                        name=f"{inst.name}-waitsplit{i}",
                        sync_info=mybir.SyncInfo(on_wait=[w], on_update=[]),
                        bass_nofuse=True,
                        ins=[],
                        outs=[],
                    )
                    nop.engine = inst.engine
                    new_insts.append(nop)
                si.on_wait[:] = keep
            new_insts.append(inst)
        blk.instructions[:] = new_insts


def _interleave(a, b, w=1.0):
    """Merge two unit lists round-robin, proportionally to their lengths.
    w > 1 front-loads list a (a's units are emitted ahead of proportional
    pace by that factor)."""
    out = []
    ia = ib = 0
    la, lb = len(a), len(b)
    while ia < la or ib < lb:
        if ib >= lb or (ia < la and ia * lb <= ib * la * w):
            out.append(a[ia])
            ia += 1
        else:
            out.append(b[ib])
            ib += 1
    return out


def build(split_waits=True):
    nc = bass.Bass("TRN2", debug=False, num_devices=NCORES)

    # x arrives pre-transposed (feature-major): bf16 for V, e4m3 for Q/K
    x_d = nc.declare_dram_parameter("x", [D, NTOK], BF, isOutput=False)
    x8_d = nc.declare_dram_parameter("x8", [D, NTOK], F8, isOutput=False)
    w_d = {}
    b_d = {}
    for nm, dt in (("wq", F8), ("wk", F8), ("wv", BF), ("wo", BF)):
        w_d[nm] = nc.declare_dram_parameter(f"{nm}_w", [D, D], dt, isOutput=False)
        b_d[nm] = nc.declare_dram_parameter(f"{nm}_b", [D], F32, isOutput=False)
    out_d = nc.declare_dram_parameter("out", [NTOK, D], F32, isOutput=True)

    with tile.TileContext(nc) as tc:
        with (
            tc.tile_pool(name="weights", bufs=1) as wpool,
            tc.tile_pool(name="consts", bufs=1) as cpool,
            tc.tile_pool(name="feat", bufs=2) as fpool,
            tc.tile_pool(name="attn", bufs=4) as apool,
            tc.tile_pool(name="outb", bufs=2) as opool,
            tc.tile_pool(name="psum", bufs=2, space="PSUM") as ppool,
        ):
            wt = {nm: [None] * KT for nm in ("wv", "wo")}
            biases = {}
            wtiles = {}
            WDT = {"wq": F8, "wk": F8, "wv": BF, "wo": BF}

            def unit_load_weight(nm, h=0, halves=1):
                """dma_start for 1/halves of the [D,D] matrix: k-tile k lands
                at cols k*D of a merged [128, KT*D] tile (contiguous runs).
                Split loads let the first k-tiles' matmuls start while the
                rest of the matrix is still in flight."""

                def f():
                    if nm not in wtiles:
                        wb = wpool.tile(
                            [128, KT * D], WDT[nm], tag=f"w_{nm}", name=f"w{nm}"
                        )
                        wtiles[nm] = wb
                        if nm in wt:
                            for k in range(KT):
                                wt[nm][k] = wb[:, k * D : (k + 1) * D]
                    wb = wtiles[nm]
                    hk = KT // halves
                    nc.sync.dma_start(
                        out=wb[:, h * hk * D : (h + 1) * hk * D].rearrange(
                            "p (k c) -> p k c", c=D
                        ),
                        in_=w_d[nm][h * hk * 128 : (h + 1) * hk * 128, :].rearrange(
                            "(k p) c -> p k c", p=128
                        ),
                    )

                return f

            def wpair(nm, kj, m):
                """[128, 2, 128] lhsT AP for DoubleRow: k-tiles (2kj, 2kj+1),
                output-feature block m."""
                wb = wtiles[nm]
                return wb[:, 2 * kj * D : (2 * kj + 2) * D].rearrange(
                    "p (k c) -> p k c", c=D
                )[:, :, m * 128 : (m + 1) * 128]

            def unit_biases_qk():
                def f():
                    # per-partition (feature-major) bias layout for q/k evac
                    for nm in ("wq", "wk"):
                        bt = cpool.tile([128, KT], F32, tag=f"{nm}_pb", name=f"{nm}_pb")
                        nc.sync.dma_start(
                            out=bt[:], in_=b_d[nm][:].rearrange("(m p) -> p m", p=128)
                        )
                        biases[nm] = bt

                return f

            def unit_biases_vo():
                """Broadcast-to-all-partitions bias tiles for v/o via a
                partition-stride-0 DMA read -- emitted late so it never
                delays the critical wq/x8/wk startup loads."""

                def f():
                    for nm in ("wv", "wo"):
                        bc = cpool.tile([128, D], F32, tag=f"{nm}_bc", name=f"{nm}_bc")
                        nc.sync.dma_start(
                            out=bc[:],
                            in_=b_d[nm][:].unsqueeze(0).broadcast_to((128, D)),
                        )
                        biases[nm] = bc

                return f

            live = {}  # per-chunk tiles handed from stage A to stage B

            def stage_a_units(ch):
                """X loads, then QKV projections for chunk ch."""
                tok0 = ch * CHUNK
                st = live.setdefault(ch, {})

                def u_x(h=0, halves=1):
                    """bf16 x chunk load (V projection operand)."""

                    def f():
                        if "xT" not in st:
                            st["xT"] = fpool.tile(
                                [128, KT * CHUNK], BF, tag="xT", name="xT"
                            )
                        hk = KT // halves
                        nc.sync.dma_start(
                            out=st["xT"][:, h * hk * CHUNK : (h + 1) * hk * CHUNK]
                            .rearrange("p (k t) -> p k t", t=CHUNK),
                            in_=x_d[
                                h * hk * 128 : (h + 1) * hk * 128,
                                tok0 : tok0 + CHUNK,
                            ].rearrange("(k p) t -> p k t", p=128),
                        )

                    return f

                def u_x8(h=0, halves=1):
                    """fp8 x chunk load (Q/K projection operand)."""

                    def f():
                        if "x8T" not in st:
                            st["x8T"] = fpool.tile(
                                [128, KT * CHUNK], F8, tag="x8T", name="x8T"
                            )
                        hk = KT // halves
                        nc.sync.dma_start(
                            out=st["x8T"][:, h * hk * CHUNK : (h + 1) * hk * CHUNK]
                            .rearrange("p (k t) -> p k t", t=CHUNK),
                            in_=x8_d[
                                h * hk * 128 : (h + 1) * hk * 128,
                                tok0 : tok0 + CHUNK,
                            ].rearrange("(k p) t -> p k t", p=128),
                        )

                    return f

                def xT(k):
                    return st["xT"][:, k * CHUNK : (k + 1) * CHUNK]

                def x8pair(kj):
                    return st["x8T"][
                        :, 2 * kj * CHUNK : (2 * kj + 2) * CHUNK
                    ].rearrange("p (k t) -> p k t", t=CHUNK)

                def u_qk(nm, dst, m):
                    def f():
                        if dst not in st:
                            st[dst] = [
                                fpool.tile(
                                    [128, CHUNK], BF, tag=f"{dst}{i}", name=f"{dst}{i}"
                                )
                                for i in range(KT)
                            ]
                        ps = ppool.tile(
                            [128, CHUNK], F32, tag="proj", bufs=2, name=f"ps_{dst}"
                        )
                        for kj in range(KT // 2):
                            nc.tensor.matmul(
                                ps[:],
                                lhsT=wpair(nm, kj, m),
                                rhs=x8pair(kj),
                                start=(kj == 0),
                                stop=(kj == KT // 2 - 1),
                                perf_mode=mybir.MatmulPerfMode.DoubleRow,
                            )
                        # DVE, not ACT: Identity activations reload the ACT
                        # function table when interleaved with Exp (~1.3us each)
                        nc.vector.tensor_scalar(
                            out=st[dst][m][:],
                            in0=ps[:],
                            scalar1=1.0 / WSCALE,
                            scalar2=biases[nm][:, m : m + 1],
                            op0=mybir.AluOpType.mult,
                            op1=mybir.AluOpType.add,
                        )

                    return f

                def u_v(t, n):
                    def f():
                        if "vaug" not in st:
                            st["vaug"] = [
                                apool.tile(
                                    [128, H * (DH + 1)], BF,
                                    tag=f"vaug{i}", name=f"vaug{i}", bufs=2,
                                )
                                for i in range(TT)
                            ]
                            for i in range(TT):
                                nc.gpsimd.memset(
                                    st["vaug"][i][:]
                                    .rearrange("p (h c) -> p h c", c=DH + 1)[:, :, DH : DH + 1],
                                    1.0,
                                )
                        ps = ppool.tile([128, CHUNK], F32, tag="proj", bufs=2, name="ps_v")
                        for k in range(KT):
                            nc.tensor.matmul(
                                ps[:],
                                lhsT=xT(k)[:, t * 128 : (t + 1) * 128],
                                rhs=wt["wv"][k][:, n * 512 : (n + 1) * 512],
                                start=(k == 0),
                                stop=(k == KT - 1),
                            )
                        nc.vector.tensor_tensor(
                            out=st["vaug"][t][:]
                            .rearrange("p (h c) -> p h c", c=DH + 1)[:, n * 8 : (n + 1) * 8, 0:DH],
                            in0=ps[:].rearrange("p (j c) -> p j c", c=DH),
                            in1=biases["wv"][:, n * 512 : (n + 1) * 512].rearrange(
                                "p (j c) -> p j c", c=DH
                            ),
                            op=mybir.AluOpType.add,
                        )

                    return f

                proj = []
                for m in range(KT):
                    proj.append(u_qk("wq", "qT", m))
                    proj.append(u_qk("wk", "kT", m))
                for t in range(TT):
                    for n in range(2):
                        proj.append(u_v(t, n))
                return {
                    "x": [u_x8(), u_x()],
                    "x8_quarters": [u_x8(qtr, 4) for qtr in range(4)],
                    "x_bf": [u_x()],
                    "q": [u_qk("wq", "qT", m) for m in range(KT)],
                    "k": [u_qk("wk", "kT", m) for m in range(KT)],
                    "v": [u_v(t, n) for t in range(TT) for n in range(2)],
                    "proj": proj,
                }

            def attn_core_units(ch, with_out=False):
                """Attention for chunk ch, software-pipelined per token-tile:
                the score pair (t+1) is emitted before ctx(t) so the ACT exp
                latency hides behind the next pair's score matmuls."""
                st = live[ch]
                es_tiles = {}

                def u_scores(u, t):
                    def f():
                        qT, kT = st["qT"], st["kT"]
                        es = apool.tile([128, 128], BF, tag="expS", name="es")
                        es_tiles[(u, t)] = es
                        for hh in (0, 1):
                            hsl = slice(hh * 64, hh * 64 + 64)
                            ps_s = ppool.tile([128, 64], F32, tag="sc", bufs=4, name=f"ps_s{hh}")
                            for bpar in (0, 1):
                                toksl = slice(u * 128 + bpar * 64, u * 128 + bpar * 64 + 64)
                                nc.tensor.matmul(
                                    ps_s[bpar * 64 : bpar * 64 + 64, :],
                                    lhsT=kT[t][hsl, toksl],
                                    rhs=qT[t][hsl, toksl],
                                    start=True,
                                    stop=True,
                                )
                            nc.scalar.activation(
                                out=es[:, hsl],
                                in_=ps_s[:],
                                func=mybir.ActivationFunctionType.Exp,
                                scale=float(SCALE),
                            )

                    return f

                def u_ctx(u, t):
                    def f():
                        if "ctx" not in st:
                            st["ctx"] = [
                                apool.tile([128, D], BF, tag=f"ctx{i}", name=f"ctx{i}", bufs=2)
                                for i in range(TT)
                            ]
                        vaug, ctx = st["vaug"], st["ctx"]
                        es = es_tiles.pop((u, t))
                        ps_c = ppool.tile([128, 130], F32, tag="cx", bufs=2, name="ps_c")
                        for bpar in (0, 1):
                            bsl = slice(bpar * 64, bpar * 64 + 64)
                            for hh in (0, 1):
                                h = 2 * t + hh
                                nc.tensor.matmul(
                                    ps_c[bsl, hh * 65 : hh * 65 + 65],
                                    lhsT=es[bsl, hh * 64 : hh * 64 + 64],
                                    rhs=vaug[u][bsl, h * 65 : (h + 1) * 65],
                                    start=True,
                                    stop=True,
                                )
                        # both heads' sumexp slots (cols 64, 129) -> one recip
                        rc = apool.tile([128, 2], F32, tag="recip", name="rc")
                        nc.vector.reciprocal(
                            rc[:].rearrange("p (h c) -> p h c", c=1),
                            ps_c[:].rearrange("p (h c) -> p h c", c=65)[:, :, DH : DH + 1],
                        )
                        for hh in (0, 1):
                            h = 2 * t + hh
                            nc.vector.tensor_scalar(
                                out=ctx[u][:, h * DH : (h + 1) * DH],
                                in0=ps_c[:, hh * 65 : hh * 65 + DH],
                                scalar1=rc[:, hh : hh + 1],
                                scalar2=None,
                                op0=mybir.AluOpType.mult,
                            )

                    return f

                def u_ctxT(u):
                    """Transpose ctx[u] [128 toks, 1024 feats] into the cT
                    blocks with ONE xbar DMA-transpose (off the PE): the 3D
                    dst AP [128, k, 128] receives transposed row k*128+pp at
                    (partition pp, block k) -- verified on hardware."""

                    def f():
                        if "cT" not in st:
                            st["cT"] = fpool.tile(
                                [128, KT * CHUNK], BF, tag="cT", name="cT", bufs=3
                            )
                        cT_blocks = st["cT"].rearrange("p (k c) -> p k c", c=CHUNK)
                        nc.sync.dma_start_transpose(
                            out=cT_blocks[:, :, u * 128 : (u + 1) * 128],
                            in_=st["ctx"][u][:],
                        )

                    return f

                groups = []
                for u in range(TT):
                    units = [u_scores(u, 0)]
                    for t in range(KT - 1):
                        units.append(u_scores(u, t + 1))
                        units.append(u_ctx(u, t))
                    units.append(u_ctx(u, KT - 1))
                    units.append(u_ctxT(u))
                    groups.append(units)
                if with_out:
                    return groups
                return [u for g in groups for u in g]

            def out_mm_units(ch):
                """Output-projection matmul + DVE bias-add into a tmp tile
                (gelu+store split out so gelus can batch on ACT)."""
                st = live[ch]

                def u_mm(t, n):
                    def f():
                        cT = st["cT"]
                        ps = ppool.tile([128, CHUNK], F32, tag="proj", bufs=2, name="ps_o")
                        for k in range(KT):
                            nc.tensor.matmul(
                                ps[:],
                                lhsT=cT[:, k * CHUNK + t * 128 : k * CHUNK + (t + 1) * 128],
                                rhs=wt["wo"][k][:, n * 512 : (n + 1) * 512],
                                start=(k == 0),
                                stop=(k == KT - 1),
                            )
                        tmp = opool.tile([128, 512], F32, tag="obuf", bufs=8, name="tmp")
                        st.setdefault("otmp", {})[(t, n)] = tmp
                        nc.vector.tensor_tensor(
                            out=tmp[:],
                            in0=ps[:],
                            in1=biases["wo"][:, n * 512 : (n + 1) * 512],
                            op=mybir.AluOpType.add,
                        )

                    return f

                return [u_mm(t, n) for t in range(TT) for n in range(2)]

            def out_act_units(ch):
                """Gelu + DRAM store for chunk ch (emitted as one batch per
                chunk: ACT reloads its function table on exp<->gelu switches)."""
                tok0 = ch * CHUNK
                st = live[ch]

                def u_act(t, n):
                    def f():
                        tmp = st["otmp"].pop((t, n))
                        og = opool.tile([128, 512], F32, tag="ogelu", name="og")
                        nc.scalar.activation(
                            out=og[:], in_=tmp[:], func=mybir.ActivationFunctionType.Gelu
                        )
                        nc.sync.dma_start(
                            out=out_d[
                                tok0 + t * 128 : tok0 + (t + 1) * 128,
                                n * 512 : (n + 1) * 512,
                            ],
                            in_=og[:],
                        )

                    return f

                return [u_act(t, n) for t in range(TT) for n in range(2)]

            # ---- emission ----
            stages = [stage_a_units(ch) for ch in range(NCH)]
            # prologue: first quarters of wq + x8(0) land first so the Q
            # projection's first DoubleRow matmuls can start while the rest
            # is still in flight; then chunk-0 projections interleaved with
            # the remaining weight loads and x(1)
            for qtr in range(4):
                unit_load_weight("wq", qtr, 4)()
                stages[0]["x8_quarters"][qtr]()
            unit_biases_qk()()
            unit_load_weight("wk")()
            for u in _interleave(stages[0]["q"], stages[0]["x_bf"] + stages[1]["x"]):
                u()
            unit_load_weight("wv")()
            unit_biases_vo()()
            for u in stages[0]["k"]:
                u()
            unit_load_weight("wo")()
            for u in stages[0]["v"]:
                u()
            # steady state: window ch emits proj(ch) + x(ch+1) + attention
            # core of ch-1 + output projection of ch-3 (the delay keeps the
            # tail full of dense work to interleave with the final attention)
            for ch in range(1, NCH):
                dense = stages[ch]["proj"]
                if ch + 1 < NCH:
                    dense = _interleave(dense, stages[ch + 1]["x"])
                if ch - 3 >= 0:
                    dense = _interleave(dense, out_mm_units(ch - 3))
                for u in _interleave(dense, attn_core_units(ch - 1)):
                    u()
                if ch - 3 >= 0:
                    # batched gelu+store drain for chunk ch-3 (2 ACT table
                    # loads per chunk instead of one per out unit)
                    for u in out_act_units(ch - 3):
                        u()
                    live.pop(ch - 3)
            # tail: the last chunk's attention, split into token-tile groups,
            # carries the three pending output projections one chunk at a
            # time — each chunk's gelu batch drains its 8 obuf tmp tiles
            # before the next chunk's bias-adds allocate them (the obuf ring
            # holds only 8; overlapping two chunks' allocations deadlocks
            # against the later-emitted gelus).
            groups = attn_core_units(NCH - 1, with_out=True)
            last_mm = out_mm_units(NCH - 1)
            # mm(7, t) reads cT block t — it must be EMITTED after ctxT(t)
            # or Tile orders the write after the read and it consumes stale
            # ring-buffer data; only t0..t2 may interleave with group 3.
            for gs, mm, act_ch in (
                (groups[0] + groups[1], out_mm_units(NCH - 3), NCH - 3),
                (groups[2], out_mm_units(NCH - 2), NCH - 2),
                (groups[3], last_mm[:6], None),
            ):
                for u in _interleave(gs, mm):
                    u()
                if act_ch is not None:
                    for u in out_act_units(act_ch):
                        u()
            for u in last_mm[6:]:
                u()
            for u in out_act_units(NCH - 1):
                u()
            live.pop(NCH - 3)
            live.pop(NCH - 2)
            live.pop(NCH - 1)

    if split_waits:
        _split_multiwait(nc)
    return nc


_NC = None


def _get_nc():
    global _NC
    if _NC is None:
        _NC = build()
    return _NC


def _make_in_maps(inputs):
    xf = np.asarray(inputs["x"], dtype=np.float32)
    full = {}
    for nm in ("wq", "wk", "wv", "wo"):
        wf = np.asarray(inputs[f"{nm}_w"], dtype=np.float32)
        if nm in ("wq", "wk"):
            # fp8 weights, pre-scaled by 1024 to stay in e4m3 normal range
            full[f"{nm}_w"] = np.ascontiguousarray(
                np.clip(wf * WSCALE, -240.0, 240.0).astype(ml_dtypes.float8_e4m3)
            )
        else:
            full[f"{nm}_w"] = np.ascontiguousarray(wf.astype(ml_dtypes.bfloat16))
        full[f"{nm}_b"] = np.ascontiguousarray(
            np.asarray(inputs[f"{nm}_b"], dtype=np.float32)
        )
    in_maps = []
    for c in range(NCORES):
        # pre-transpose to feature-major [D, NTOK] so no on-chip x transpose
        xc = np.ascontiguousarray(xf[c * BL : (c + 1) * BL].reshape(NTOK, D).T)
        m = {
            "x": xc.astype(ml_dtypes.bfloat16),
            "x8": np.clip(xc, -240.0, 240.0).astype(ml_dtypes.float8_e4m3),
        }
        m.update(full)
        in_maps.append(m)
    return in_maps


def kernel(**inputs):
    nc = _get_nc()
    res = run_bass_kernel_spmd(
        nc, _make_in_maps(inputs), core_ids=list(range(NCORES))
    ).results
    parts = [res[c]["out"].reshape(BL, 8, 8, D) for c in range(NCORES)]
    return np.concatenate(parts, axis=0)


def kernel_profiled(**inputs):
    """Like kernel() but requests an NTFF trace; returns (out, exec_time_ns, raw)."""
    nc = _get_nc()
    r = run_bass_kernel_spmd(
        nc, _make_in_maps(inputs), core_ids=list(range(NCORES)), trace=True
    )
    parts = [r.results[c]["out"].reshape(BL, 8, 8, D) for c in range(NCORES)]
    return np.concatenate(parts, axis=0), r.exec_time_ns, r
